# revision 3
# baseline (speedup 1.0000x reference)
"""DetectionLayer decode kernel for Trainium2 (Bass/Tile), 8-core SPMD.

Computes, for inputs [N, 85] and anchors [N, 4] (N = 2,000,000):
    cond    = inputs[:, 5] > 0.5
    pred_yx = inputs[:, :2] * anchors[:, 2:4] + anchors[:, :2]
    pred_hw = exp(inputs[:, 2:4]) * anchors[:, 2:4]
    out     = where(cond, concat([pred_yx, pred_hw, inputs[:, 4:]]), 0)

Sharding: row dimension split into 8 equal-shape overlapping windows
(window R rows, stride S; 7*S + R == N) so every core runs the same NEFF
on a 128*K-row-aligned shard with no host-side padding copies.
"""
import sys

sys.path.insert(0, "/opt/trn_rl_repo")

import numpy as np

import concourse.bacc as bacc
import concourse.mybir as mybir
from concourse.bass_utils import run_bass_kernel_spmd
from concourse.tile import TileContext

N = 2_000_000
C = 85
N_CORES = 8
P = 128           # SBUF partitions
K = 16            # anchor rows per partition per tile
TILE_ROWS = P * K  # 2048
T = 123           # tiles per core
R = T * TILE_ROWS  # 251,904 rows per core window
S = 249_728        # window stride; 7*S + R == N
THR = 0.5

assert 7 * S + R == N and S % P == 0 and S <= R

_NC_CACHE = None


def _build_module(n_tiles=T):
    rows = n_tiles * TILE_ROWS
    nc = bacc.Bacc("TRN2", target_bir_lowering=False, debug=False)
    inp = nc.dram_tensor("inputs", [rows, C], mybir.dt.float32, kind="ExternalInput")
    anc = nc.dram_tensor("anchors", [rows, 4], mybir.dt.float32, kind="ExternalInput")
    out = nc.dram_tensor("out", [rows, C], mybir.dt.float32, kind="ExternalOutput")

    # Row r = t*TILE_ROWS + p*K + g  ->  tile t, partition p, group g.
    iv = inp.ap().rearrange("(t p g) c -> t p (g c)", p=P, g=K)  # [nt, 128, K*C]
    ov = out.ap().rearrange("(t p g) c -> t p (g c)", p=P, g=K)
    # All anchors resident in SBUF: [128, nt*K*4], tile t at cols [t*K*4,(t+1)*K*4)
    av_all = anc.ap().rearrange("(t p g) c -> t p (g c)", p=P, g=K).transpose([1, 0, 2])

    with TileContext(nc) as tc:
        with tc.tile_pool(name="anc", bufs=1) as apool, \
             tc.tile_pool(name="sbuf", bufs=4) as pool:
            anc_all = apool.tile([P, n_tiles * K * 4], mybir.dt.float32, tag="anc_all")
            nc.sync.dma_start(out=anc_all[:], in_=av_all)
            for t in range(n_tiles):
                in_t = pool.tile([P, K * C], mybir.dt.float32, tag="in")
                out_t = pool.tile([P, K * C], mybir.dt.float32, tag="out")
                am_t = pool.tile([P, K * 4], mybir.dt.float32, tag="am")
                ex_t = pool.tile([P, K * 2], mybir.dt.float32, tag="ex")

                nc.sync.dma_start(out=in_t[:], in_=iv[t])

                ing = in_t[:].rearrange("p (g c) -> p g c", c=C)
                outg = out_t[:].rearrange("p (g c) -> p g c", c=C)
                ang = anc_all[:, t * K * 4:(t + 1) * K * 4].rearrange(
                    "p (g c) -> p g c", c=4)
                amg = am_t[:].rearrange("p (g c) -> p g c", c=4)
                score = ing[:, :, 5:6]

                # out = (score > THR) * in ; cols 0..3 recomputed below
                nc.vector.scalar_tensor_tensor(
                    out=outg,
                    in0=score.broadcast_to([P, K, C]),
                    scalar=THR,
                    in1=ing,
                    op0=mybir.AluOpType.is_gt,
                    op1=mybir.AluOpType.mult,
                )
                # masked anchors: am = (score > THR) * anchors
                nc.vector.scalar_tensor_tensor(
                    out=amg,
                    in0=score.broadcast_to([P, K, 4]),
                    scalar=THR,
                    in1=ang,
                    op0=mybir.AluOpType.is_gt,
                    op1=mybir.AluOpType.mult,
                )
                # ex = exp(in[:, 2:4]) on the scalar engine (off DVE's critical path)
                nc.scalar.activation(
                    ex_t[:].rearrange("p (g c) -> p g c", c=2),
                    ing[:, :, 2:4],
                    mybir.ActivationFunctionType.Exp,
                )
                # out[:, 0:2] = in[:, 0:2] * am[:, 2:4] + am[:, 0:2]
                nc.vector.tensor_mul(outg[:, :, 0:2], ing[:, :, 0:2], amg[:, :, 2:4])
                nc.vector.tensor_add(outg[:, :, 0:2], outg[:, :, 0:2], amg[:, :, 0:2])
                # out[:, 2:4] = ex * am[:, 2:4]
                nc.vector.tensor_mul(
                    outg[:, :, 2:4],
                    ex_t[:].rearrange("p (g c) -> p g c", c=2),
                    amg[:, :, 2:4],
                )

                nc.sync.dma_start(out=ov[t], in_=out_t[:])
    nc.compile()
    return nc


def _get_module():
    global _NC_CACHE
    if _NC_CACHE is None:
        _NC_CACHE = _build_module()
    return _NC_CACHE


def _run(inputs, anchors, **spmd_kwargs):
    inputs = np.ascontiguousarray(np.asarray(inputs, dtype=np.float32))
    anchors = np.ascontiguousarray(np.asarray(anchors, dtype=np.float32))
    assert inputs.shape == (N, C) and anchors.shape == (N, 4)

    nc = _get_module()
    in_maps = [
        {"inputs": inputs[i * S : i * S + R], "anchors": anchors[i * S : i * S + R]}
        for i in range(N_CORES)
    ]
    res = run_bass_kernel_spmd(nc, in_maps, core_ids=list(range(N_CORES)), **spmd_kwargs)

    out = np.empty((N, C), dtype=np.float32)
    for i in range(N_CORES - 1):
        out[i * S : (i + 1) * S] = res.results[i]["out"][:S]
    out[(N_CORES - 1) * S :] = res.results[N_CORES - 1]["out"]
    return out, res


def kernel(inputs, anchors):
    out, _ = _run(inputs, anchors)
    return out


if __name__ == "__main__":
    rng = np.random.default_rng(0)
    x = rng.random((N, C), dtype=np.float32)
    a = rng.random((N, 4), dtype=np.float32)
    y = kernel(x, a)
    print("ran ok", y.shape, y.dtype)


# revision 6
# speedup vs baseline: 1.1291x; 1.1291x over previous
"""DetectionLayer decode kernel for Trainium2 (Bass/Tile), 8-core SPMD.

Computes, for inputs [N, 85] and anchors [N, 4] (N = 2,000,000):
    cond    = inputs[:, 5] > 0.5
    pred_yx = inputs[:, :2] * anchors[:, 2:4] + anchors[:, :2]
    pred_hw = exp(inputs[:, 2:4]) * anchors[:, 2:4]
    out     = where(cond, concat([pred_yx, pred_hw, inputs[:, 4:]]), 0)

Sharding: row dimension split into 8 equal-shape overlapping windows
(window R rows, stride S; 7*S + R == N) so every core runs the same NEFF
on a 128*K-row-aligned shard with no host-side padding copies.
"""
import sys

sys.path.insert(0, "/opt/trn_rl_repo")

import numpy as np

import concourse.bacc as bacc
import concourse.mybir as mybir
from concourse.bass_utils import run_bass_kernel_spmd
from concourse.tile import TileContext

N = 2_000_000
C = 85
N_CORES = 8
P = 128           # SBUF partitions
K = 24            # anchor rows per partition per tile
TILE_ROWS = P * K  # 3072
T = 82            # tiles per core
R = T * TILE_ROWS  # 251,904 rows per core window
S = 249_728        # window stride; 7*S + R == N
THR = 0.5

assert 7 * S + R == N and S % P == 0 and S <= R

_NC_CACHE = None


def _build_module(n_tiles=T):
    rows = n_tiles * TILE_ROWS
    nc = bacc.Bacc("TRN2", target_bir_lowering=False, debug=False)
    inp = nc.dram_tensor("inputs", [rows, C], mybir.dt.float32, kind="ExternalInput")
    anc = nc.dram_tensor("anchors", [rows, 4], mybir.dt.float32, kind="ExternalInput")
    out = nc.dram_tensor("out", [rows, C], mybir.dt.float32, kind="ExternalOutput")

    # Row r = t*TILE_ROWS + p*K + g  ->  tile t, partition p, group g.
    iv = inp.ap().rearrange("(t p g) c -> t p (g c)", p=P, g=K)  # [nt, 128, K*C]
    ov = out.ap().rearrange("(t p g) c -> t p (g c)", p=P, g=K)
    # All anchors resident in SBUF: [128, nt*K*4], tile t at cols [t*K*4,(t+1)*K*4)
    av_all = anc.ap().rearrange("(t p g) c -> t p (g c)", p=P, g=K).transpose([1, 0, 2])

    with TileContext(nc) as tc:
        with tc.tile_pool(name="anc", bufs=1) as apool, \
             tc.tile_pool(name="sbuf", bufs=4) as pool:
            anc_all = apool.tile([P, n_tiles * K * 4], mybir.dt.float32, tag="anc_all")
            nc.sync.dma_start(out=anc_all[:], in_=av_all)
            for t in range(n_tiles):
                in_t = pool.tile([P, K * C], mybir.dt.float32, tag="in")
                out_t = pool.tile([P, K * C], mybir.dt.float32, tag="out")
                am_t = pool.tile([P, K * 4], mybir.dt.float32, tag="am")

                nc.sync.dma_start(out=in_t[:], in_=iv[t])

                ing = in_t[:].rearrange("p (g c) -> p g c", c=C)
                outg = out_t[:].rearrange("p (g c) -> p g c", c=C)
                ang = anc_all[:, t * K * 4:(t + 1) * K * 4].rearrange(
                    "p (g c) -> p g c", c=4)
                amg = am_t[:].rearrange("p (g c) -> p g c", c=4)
                score = ing[:, :, 5:6]

                # out = (score > THR) * in ; cols 0..3 recomputed below
                nc.vector.scalar_tensor_tensor(
                    out=outg,
                    in0=score.broadcast_to([P, K, C]),
                    scalar=THR,
                    in1=ing,
                    op0=mybir.AluOpType.is_gt,
                    op1=mybir.AluOpType.mult,
                )
                # masked anchors: am = (score > THR) * anchors
                nc.vector.scalar_tensor_tensor(
                    out=amg,
                    in0=score.broadcast_to([P, K, 4]),
                    scalar=THR,
                    in1=ang,
                    op0=mybir.AluOpType.is_gt,
                    op1=mybir.AluOpType.mult,
                )
                # in[:, 2:4] = exp(in[:, 2:4]) in place on the scalar engine
                # (after the big masked copy has read the raw values)
                nc.scalar.activation(
                    ing[:, :, 2:4],
                    ing[:, :, 2:4],
                    mybir.ActivationFunctionType.Exp,
                )
                # out[:, 0:4] = [in_yx, exp(in_hw)] * [am_hw, am_hw]
                nc.vector.tensor_mul(
                    outg[:, :, 0:4].rearrange("p g (a b) -> p g a b", b=2),
                    ing[:, :, 0:4].rearrange("p g (a b) -> p g a b", b=2),
                    amg[:, :, 2:4].unsqueeze(2).broadcast_to([P, K, 2, 2]),
                )
                # out[:, 0:2] += am_yx
                nc.vector.tensor_add(outg[:, :, 0:2], outg[:, :, 0:2], amg[:, :, 0:2])

                nc.scalar.dma_start(out=ov[t], in_=out_t[:])
    nc.compile()
    return nc


def _get_module():
    global _NC_CACHE
    if _NC_CACHE is None:
        _NC_CACHE = _build_module()
    return _NC_CACHE


def _run(inputs, anchors, **spmd_kwargs):
    inputs = np.ascontiguousarray(np.asarray(inputs, dtype=np.float32))
    anchors = np.ascontiguousarray(np.asarray(anchors, dtype=np.float32))
    assert inputs.shape == (N, C) and anchors.shape == (N, 4)

    nc = _get_module()
    in_maps = [
        {"inputs": inputs[i * S : i * S + R], "anchors": anchors[i * S : i * S + R]}
        for i in range(N_CORES)
    ]
    res = run_bass_kernel_spmd(nc, in_maps, core_ids=list(range(N_CORES)), **spmd_kwargs)

    out = np.empty((N, C), dtype=np.float32)
    for i in range(N_CORES - 1):
        out[i * S : (i + 1) * S] = res.results[i]["out"][:S]
    out[(N_CORES - 1) * S :] = res.results[N_CORES - 1]["out"]
    return out, res


def kernel(inputs, anchors):
    out, _ = _run(inputs, anchors)
    return out


if __name__ == "__main__":
    rng = np.random.default_rng(0)
    x = rng.random((N, C), dtype=np.float32)
    a = rng.random((N, 4), dtype=np.float32)
    y = kernel(x, a)
    print("ran ok", y.shape, y.dtype)
